# revision 4
# baseline (speedup 1.0000x reference)
"""Trainium2 Bass kernel for nn_DecoderCell (B=128,N=512,C=4,T=128,D=128,H=8).

Pure data-parallel over batch B across 8 NeuronCores (16 b/core).

v2 design notes:
- Q1 (query projection incl. step context + fixed query) precomputed on host,
  pre-scaled by A/sqrt(DH) with A = 2^7/ln2, head-permuted into two pass
  tiles (heads 0-3 / 4-7), so score PSUM holds A*s directly.
- Softmax exp is a Schraudolph bit-trick: one tensor_tensor(psum + maskB ->
  int16) per score tile; maskB carries the exponent bias 16256 for kept
  entries and -40960 for masked ones, which saturates the int16 convert to
  -32768 = bf16 -0.0 (exact zero contribution to U and Z).
- Score tiles split between DVE (TT direct, 1x) and ScalarE (Identity copy
  to bf16, then DVE TT at 2x) to balance the PSUM-read bottleneck across
  both PSUM-capable engines. GpSimd (no PSUM access) takes SBUF-only work:
  vaug memsets, final-stage Schraudolph exp and output scale/bias.
- Z rows come free from a ones-augmented V; broadcast via one select-matmul
  (selZ) straight from the u2 copy, then reciprocal_approx_fast + multiply.
- Final stage: tanh (ScalarE) -> +mask (DVE 2x) -> Schraudolph exp (GpSimd)
  -> row-sum Z (DVE) -> log via exponent-bits affine -> scale/bias out.
  lnZ affine includes the +0.0397 mean-log Schraudolph correction.
"""
import numpy as np
import ml_dtypes

D = 128
N = 512
C = 4
T = 128
Q = T * C          # 512 queries per batch, q = t*C + c
H = 8
DH = 16
NB = 16            # batches per core
NCORES = 8

A = 184.6649652337873       # 2^7/ln2
BEXP = 16256.0              # 127*128: bf16 exponent bias << 7
MASKB = -40960.0            # BEXP + mask -> saturate int16 -> bf16 -0.0
MNEG = -1.0e8               # final-stage mask (x10 ~= -1e9), bf16
A10 = 1846.649652337873     # 10*A for final exp
LN2 = 0.6931471805599453
SCHRAUD_MEANLOG = 0.0572809000084; # E[log2((1+f)/2^f)], f~U(0,1)

HA = [0, 1, 2, 3]
HB = [4, 5, 6, 7]
BF = ml_dtypes.bfloat16

# score-tile units 0..15: (pass, j, A/B) -> u = (pi*4+j)*2 + (0 if A else 1)
# s-units go through ScalarE copy + DVE 2x TT; rest are DVE 1x TT.
S_UNITS = frozenset({1, 3, 5, 8, 10, 12, 14})


def _perm_cols(W, heads):
    """Columns of W[*,128] so head g sits at cols 32g..32g+15, zeros after."""
    out = np.zeros_like(W)
    for g, h in enumerate(heads):
        out[:, 32 * g:32 * g + 16] = W[:, 16 * h:16 * h + 16]
    return out


def _perm_rows(W, heads):
    out = np.zeros_like(W)
    for g, h in enumerate(heads):
        out[32 * g:32 * g + 16, :] = W[16 * h:16 * h + 16, :]
    return out


def _host_prep(inputs):
    """Full-input numpy prep -> per-core input dicts."""
    ne = np.ascontiguousarray(inputs["node_embeddings"], np.float32)
    ge = np.ascontiguousarray(inputs["graph_embedding"], np.float32)
    sc = np.ascontiguousarray(inputs["step_context"], np.float32)
    mask = np.asarray(inputs["mask"])
    B = ne.shape[0]

    net = ne.transpose(0, 2, 1).astype(BF)                        # [B,D,N]

    # ---- host Q1: (sc @ Wq_step + ge @ Wq_fixed) * (A/sqrt(DH)) ----
    Wqs = np.asarray(inputs["Wq_step"], np.float32)               # [129,128]
    Wqf = np.asarray(inputs["Wq_fixed"], np.float32)
    scq = sc[:, :, :, 0, :]                                       # [T,B,C,129]
    q1 = scq.reshape(-1, D + 1) @ Wqs                             # [T*B*C,128]
    q1 = q1.reshape(T, B, C, D) + (ge @ Wqf)[None, :, None, :]
    q1 *= np.float32(A / np.sqrt(DH))
    # [B, d, q] with q = t*C + c
    q1 = q1.transpose(1, 3, 0, 2).reshape(B, D, Q)
    q1ta = np.zeros((B, 128, Q), np.float32)
    q1tb = np.zeros((B, 128, Q), np.float32)
    for g in range(4):
        q1ta[:, 32 * g:32 * g + 16] = q1[:, 16 * HA[g]:16 * HA[g] + 16]
        q1tb[:, 32 * g:32 * g + 16] = q1[:, 16 * HB[g]:16 * HB[g] + 16]
    q1ta = q1ta.astype(BF)
    q1tb = q1tb.astype(BF)

    # ---- masks ----
    m4 = mask[:, :, :, 0, :]                                      # [T,B,C,N]
    # attention maskB: [B, 128 n', 4 j, Q] bf16, n = 128j + n'
    mt = m4.transpose(1, 3, 0, 2).reshape(B, N, Q)
    mb = np.where(mt, np.float32(MASKB), np.float32(BEXP))
    maskB = np.ascontiguousarray(
        mb.reshape(B, 4, 128, Q).transpose(0, 2, 1, 3)).astype(BF)
    # final-stage mask [B, 128 q', 4 i, N] bf16, value MNEG
    mn = m4.transpose(1, 0, 2, 3).reshape(B, Q, N)
    mkneg = np.ascontiguousarray(
        (mn.reshape(B, 4, 128, N).transpose(0, 2, 1, 3).astype(np.float32)
         * np.float32(MNEG))).astype(BF)

    # ---- weights ----
    Wk1 = np.asarray(inputs["Wk1"], np.float32)
    bfw = lambda x: np.ascontiguousarray(x).astype(BF)
    selZ = np.zeros((128, 128), np.float32)
    for p in range(128):
        selZ[32 * (p // 32) + 16, p] = 1.0
    weights = {
        "wk1a": bfw(_perm_cols(Wk1, HA)), "wk1b": bfw(_perm_cols(Wk1, HB)),
        "wv": bfw(inputs["Wv"]),
        "wk2s": bfw(np.asarray(inputs["Wk2"], np.float32)
                    / np.float32(np.sqrt(D))),
        "wouta": bfw(_perm_rows(np.asarray(inputs["Wout"], np.float32), HA)),
        "woutb": bfw(_perm_rows(np.asarray(inputs["Wout"], np.float32), HB)),
        "selz": selZ.astype(BF),
        # p4sel[q', t'] = 1 iff q'//4 == t'  (c-sum per step)
        "p4sel": np.stack([
            ((np.arange(128) // 4) == tp).astype(np.float32)
            for tp in range(32)], axis=1).astype(BF),
        # p432[t', q'] = -1 iff q'//4 == t'  (negated lnZ broadcast)
        "p432": (-np.stack([
            ((np.arange(128) // 4) == tp).astype(np.float32)
            for tp in range(32)])).astype(BF),
    }

    core_ins = []
    for ci in range(NCORES):
        b0 = ci * NB
        sl = slice(b0, b0 + NB)
        m = dict(weights)
        m.update({
            "net": np.ascontiguousarray(net[sl]),
            "q1ta": np.ascontiguousarray(q1ta[sl]),
            "q1tb": np.ascontiguousarray(q1tb[sl]),
            "maskb": np.ascontiguousarray(maskB[sl]),
            "mkneg": np.ascontiguousarray(mkneg[sl]),
        })
        core_ins.append(m)
    return core_ins


def build_kernel(nb=NB):
    import concourse.bacc as bacc
    import concourse.mybir as mybir
    import concourse.tile as tile

    dt = mybir.dt
    f32, bf16, i16, i32 = dt.float32, dt.bfloat16, dt.int16, dt.int32
    AF = mybir.ActivationFunctionType
    OP = mybir.AluOpType

    nc = bacc.Bacc("TRN2", target_bir_lowering=False, debug=False,
                   num_devices=NCORES)

    din = {}
    def dram(name, shape, dtype, kind="ExternalInput"):
        din[name] = nc.dram_tensor(name, shape, dtype, kind=kind)
        return din[name]

    net = dram("net", [nb, D, N], bf16)
    q1ta = dram("q1ta", [nb, 128, Q], bf16)
    q1tb = dram("q1tb", [nb, 128, Q], bf16)
    maskb = dram("maskb", [nb, 128, 4, Q], bf16)
    mkneg = dram("mkneg", [nb, 128, 4, N], bf16)
    for w in ("wk1a", "wk1b", "wv", "wk2s", "wouta", "woutb", "selz",
              "p4sel", "p432"):
        shape = ([128, 32] if w == "p4sel" else
                 ([32, 128] if w == "p432" else [128, 128]))
        dram(w, shape, bf16)
    # device layout [q'=(t', c), b, i, n]; host reassembles t = 32*i + t'
    out = dram("out", [128, nb, 4, N], bf16, kind="ExternalOutput")

    with tile.TileContext(nc) as tc:
        from contextlib import ExitStack
        with ExitStack() as ctx:
            wp = ctx.enter_context(tc.tile_pool(name="wp", bufs=1))
            io = ctx.enter_context(tc.tile_pool(name="io", bufs=2))
            wk = ctx.enter_context(tc.tile_pool(name="wk", bufs=2))
            big = ctx.enter_context(tc.tile_pool(name="big", bufs=2))
            sm = ctx.enter_context(tc.tile_pool(name="sm", bufs=2))
            # PSUM: pss 2 x [128,2,512] (2 banks each) + pu 4 x [128,512]
            pss = ctx.enter_context(tc.tile_pool(name="pss", bufs=2, space="PSUM"))
            pu = ctx.enter_context(tc.tile_pool(name="pu", bufs=4, space="PSUM"))

            W = {}
            for wn in ("wk1a", "wk1b", "wv", "wk2s", "wouta", "woutb",
                       "selz", "p4sel", "p432"):
                t = wp.tile(list(din[wn].shape), din[wn].dtype, tag=f"w_{wn}")
                nc.sync.dma_start(out=t, in_=din[wn][:, :])
                W[wn] = t

            for b in range(nb):
                # ---------- DMA loads ----------
                net_t = io.tile([D, N], bf16, tag="net")
                nc.sync.dma_start(out=net_t, in_=net[b])
                q1a_t = io.tile([128, Q], bf16, tag="q1a")
                nc.sync.dma_start(out=q1a_t, in_=q1ta[b])
                q1b_t = io.tile([128, Q], bf16, tag="q1b")
                nc.sync.dma_start(out=q1b_t, in_=q1tb[b])
                mb_t = io.tile([128, 4, Q], bf16, tag="maskb")
                nc.sync.dma_start(out=mb_t, in_=maskb[b])
                mkn_t = io.tile([128, 4, N], bf16, tag="mkneg")
                nc.sync.dma_start(out=mkn_t, in_=mkneg[b])

                # ---------- projections ----------
                def proj_to_sbuf(wtile, rhs, tag):
                    ps = pu.tile([128, N], f32, tag="pu")
                    nc.tensor.matmul(ps, lhsT=wtile, rhs=rhs)
                    sb = wk.tile([128, N], bf16, tag=tag)
                    nc.scalar.copy(sb, ps)
                    return sb

                k1ta = proj_to_sbuf(W["wk1a"], net_t, "k1ta")
                k1tb = proj_to_sbuf(W["wk1b"], net_t, "k1tb")
                k2t = proj_to_sbuf(W["wk2s"], net_t, "k2t")

                # V: psum [128, 4, 128] then one permuted copy into vaug
                pv = pu.tile([128, 4, 128], f32, tag="pu")
                for j in range(4):
                    nc.tensor.matmul(
                        pv[:, j, :], lhsT=net_t[:, 128 * j:128 * (j + 1)],
                        rhs=W["wv"])
                vaug = wk.tile([128, 2, 4, 128], bf16, tag="vaug")
                nc.gpsimd.memset(vaug, 0.0)
                nc.gpsimd.memset(
                    vaug.rearrange("p x j (g c) -> p x j g c", g=4)
                    [:, :, :, :, 16:17], 1.0)
                nc.scalar.copy(
                    vaug.rearrange("p x j (g c) -> p x j g c", g=4)
                    [:, :, :, :, 0:16],
                    pv.rearrange("p j (x g r) -> p x j g r", x=2, g=4))

                # ---------- attention ----------
                psu = {}
                for pi, (k1t, q1t) in enumerate(
                        ((k1ta, q1a_t), (k1tb, q1b_t))):
                    psu[pi] = pu.tile([128, Q], f32, name=f"psu{pi}",
                                      tag="pu")
                    for j in range(4):
                        psA = pss.tile([128, 2, Q], f32, tag="sc")
                        psB = pss.tile([128, 2, Q], f32, tag="sc")
                        for g in range(4):
                            ps2 = psA if g < 2 else psB
                            sl = slice(32 * g, 32 * g + 16)
                            nc.tensor.matmul(
                                ps2[:, g % 2, :],
                                lhsT=k1t[sl, 128 * j:128 * (j + 1)],
                                rhs=q1t[sl, :], start=True, stop=True,
                                tile_position=(32 * g, 0),
                                skip_group_check=True)
                        mbb = mb_t[:, j, None, :].broadcast_to([128, 2, Q])
                        for half, ps2 in ((0, psA), (1, psB)):
                            u = (pi * 4 + j) * 2 + half
                            es = big.tile([128, 2, Q], i16, tag="es")
                            if u in S_UNITS:
                                sp = big.tile([128, 2, Q], bf16, tag="sp")
                                nc.scalar.copy(sp, ps2)
                                nc.vector.tensor_tensor(es, sp, mbb, OP.add)
                            else:
                                nc.vector.tensor_tensor(es, ps2, mbb, OP.add)
                            em = es.bitcast(bf16)
                            for g2 in (0, 1):
                                g = half * 2 + g2
                                nc.tensor.matmul(
                                    psu[pi][32 * g:32 * g + 32, :],
                                    lhsT=vaug[:, pi, j, 32 * g:32 * g + 32],
                                    rhs=em[:, g2, :],
                                    start=(j == 0), stop=(j == 3),
                                    tile_position=(0, 32 * g),
                                    skip_group_check=True)

                # ---------- normalize + Q2 ----------
                un = {}
                for pi in range(2):
                    u2 = wk.tile([128, Q], bf16, tag=f"u2{pi}")
                    nc.scalar.copy(u2, psu[pi])
                    zbc = pu.tile([128, Q], f32, tag="pu")
                    nc.tensor.matmul(zbc, lhsT=W["selz"], rhs=u2)
                    rinv = big.tile([128, Q], f32, tag=f"rinv{pi}")
                    nc.vector.reciprocal_approx_fast(out=rinv, in_=zbc)
                    u_n = wk.tile([128, Q], bf16, tag=f"un{pi}")
                    nc.vector.tensor_tensor(u_n, u2, rinv, OP.mult)
                    un[pi] = u_n

                pq2 = pu.tile([128, Q], f32, tag="pu")
                nc.tensor.matmul(pq2, lhsT=W["wouta"], rhs=un[0],
                                 start=True, stop=False)
                nc.tensor.matmul(pq2, lhsT=W["woutb"], rhs=un[1],
                                 start=False, stop=True)
                q2t = wk.tile([128, Q], bf16, tag="q2t")
                nc.scalar.copy(q2t, pq2)

                # ---------- logits / final ----------
                th = big.tile([128, 4, N], bf16, tag="th")
                for ii in range(2):
                    pl = pss.tile([128, 2, N], f32, tag="sc")
                    for i2 in range(2):
                        i = 2 * ii + i2
                        nc.tensor.matmul(
                            pl[:, i2, :],
                            lhsT=q2t[:, 128 * i:128 * (i + 1)], rhs=k2t)
                    nc.scalar.activation(th[:, 2 * ii:2 * ii + 2, :], pl,
                                         AF.Tanh)

                s1 = big.tile([128, 4, N], bf16, tag="s1")
                nc.vector.tensor_tensor(s1, th, mkn_t, OP.add)
                es2 = big.tile([128, 4, N], i16, tag="es2")
                nc.gpsimd.tensor_scalar(es2, s1, A10, BEXP, OP.mult, OP.add)
                zf = sm.tile([128, 4], f32, tag="zf")
                nc.vector.tensor_reduce(zf, es2.bitcast(bf16),
                                        mybir.AxisListType.X, OP.add)
                zb = sm.tile([128, 4], bf16, tag="zb")
                nc.vector.tensor_copy(zb, zf)
                pmisc = pu.tile([128, N], f32, tag="pu")
                nc.tensor.matmul(pmisc[0:32, 0:4], lhsT=W["p4sel"], rhs=zb)
                zi = sm.tile([32, 4], f32, tag="zi")
                nc.vector.tensor_copy(zi, pmisc[0:32, 0:4].bitcast(i32))
                lnzb = sm.tile([32, 4], bf16, tag="lnzb")
                nc.vector.tensor_scalar(
                    lnzb, zi, LN2 / (1 << 23),
                    -(127.043 + SCHRAUD_MEANLOG) * LN2, OP.mult, OP.add)
                nc.tensor.matmul(pmisc[:, 4:8], lhsT=W["p432"], rhs=lnzb)
                bias = sm.tile([128, 4], f32, tag="bias")
                nc.vector.tensor_copy(bias, pmisc[:, 4:8])

                out_sb = big.tile([128, 4, N], bf16, tag="outsb")
                for i in range(4):
                    nc.gpsimd.tensor_scalar(
                        out_sb[:, i, :], s1[:, i, :], 10.0,
                        bias[:, i:i + 1], OP.mult, OP.add)
                nc.sync.dma_start(out=out[:, b, :, :], in_=out_sb)

    nc.compile()
    return nc


_CACHED = None


def _get_nc():
    global _CACHED
    if _CACHED is None:
        _CACHED = build_kernel()
    return _CACHED


def kernel(**inputs):
    from concourse.bass_utils import run_bass_kernel_spmd

    core_ins = _host_prep(inputs)
    nc = _get_nc()
    res = run_bass_kernel_spmd(nc, core_ins, core_ids=list(range(NCORES)))
    outs = [_unscramble(r["out"]) for r in res.results]   # each [T, NB, 2048]
    return np.concatenate(outs, axis=1)                   # [T, B, 2048]


def _unscramble(dev):
    """Device [128 q'=(t',c), nb, 4 i, 512 n] -> [T, nb, C*N], t=32i+t'."""
    nb = dev.shape[1]
    return (dev.astype(np.float32)
            .reshape(32, C, nb, 4, N)
            .transpose(3, 0, 2, 1, 4)
            .reshape(T, nb, C * N))


# revision 6
# speedup vs baseline: 1.2203x; 1.2203x over previous
"""Trainium2 Bass kernel for nn_DecoderCell (B=128,N=512,C=4,T=128,D=128,H=8).

Pure data-parallel over batch B across 8 NeuronCores (16 b/core).

v2 design notes:
- Q1 (query projection incl. step context + fixed query) precomputed on host,
  pre-scaled by A/sqrt(DH) with A = 2^7/ln2, head-permuted into two pass
  tiles (heads 0-3 / 4-7), so score PSUM holds A*s directly.
- Softmax exp is a Schraudolph bit-trick: one tensor_tensor(psum + maskB ->
  int16) per score tile; maskB carries the exponent bias 16256 for kept
  entries and -40960 for masked ones, which saturates the int16 convert to
  -32768 = bf16 -0.0 (exact zero contribution to U and Z).
- Score tiles split between DVE (TT direct, 1x) and ScalarE (Identity copy
  to bf16, then DVE TT at 2x) to balance the PSUM-read bottleneck across
  both PSUM-capable engines. GpSimd (no PSUM access) takes SBUF-only work:
  vaug memsets, final-stage Schraudolph exp and output scale/bias.
- Z rows come free from a ones-augmented V; broadcast via one select-matmul
  (selZ) straight from the u2 copy, then reciprocal_approx_fast + multiply.
- Final stage: tanh (ScalarE) -> +mask (DVE 2x) -> Schraudolph exp (GpSimd)
  -> row-sum Z (DVE) -> log via exponent-bits affine -> scale/bias out.
  lnZ affine includes the +0.0397 mean-log Schraudolph correction.
"""
import numpy as np
import ml_dtypes

D = 128
N = 512
C = 4
T = 128
Q = T * C          # 512 queries per batch, q = t*C + c
H = 8
DH = 16
NB = 16            # batches per core
NCORES = 8

A = 184.6649652337873       # 2^7/ln2
BEXP = 16256.0              # 127*128: bf16 exponent bias << 7
MASKB = -40960.0            # BEXP + mask -> saturate int16 -> bf16 -0.0
MNEG = -1.0e8               # final-stage mask (x10 ~= -1e9), bf16
A10 = 1846.649652337873     # 10*A for final exp
LN2 = 0.6931471805599453
SCHRAUD_MEANLOG = 0.0572809000084; # E[log2((1+f)/2^f)], f~U(0,1)

HA = [0, 1, 2, 3]
HB = [4, 5, 6, 7]
BF = ml_dtypes.bfloat16

# score-tile units 0..15: (pass, j, A/B) -> u = (pi*4+j)*2 + (0 if A else 1)
# s-units go through ScalarE copy + DVE 2x TT; rest are DVE 1x TT.
S_UNITS = frozenset({1, 3, 6, 8, 11, 13})


def _perm_cols(W, heads):
    """Columns of W[*,128] so head g sits at cols 32g..32g+15, zeros after."""
    out = np.zeros_like(W)
    for g, h in enumerate(heads):
        out[:, 32 * g:32 * g + 16] = W[:, 16 * h:16 * h + 16]
    return out


def _perm_rows(W, heads):
    out = np.zeros_like(W)
    for g, h in enumerate(heads):
        out[32 * g:32 * g + 16, :] = W[16 * h:16 * h + 16, :]
    return out


def _host_prep(inputs):
    """Full-input numpy prep -> per-core input dicts."""
    ne = np.ascontiguousarray(inputs["node_embeddings"], np.float32)
    ge = np.ascontiguousarray(inputs["graph_embedding"], np.float32)
    sc = np.ascontiguousarray(inputs["step_context"], np.float32)
    mask = np.asarray(inputs["mask"])
    B = ne.shape[0]

    net = ne.transpose(0, 2, 1).astype(BF)                        # [B,D,N]

    # ---- host Q1: (sc @ Wq_step + ge @ Wq_fixed) * (A/sqrt(DH)) ----
    Wqs = np.asarray(inputs["Wq_step"], np.float32)               # [129,128]
    Wqf = np.asarray(inputs["Wq_fixed"], np.float32)
    scq = sc[:, :, :, 0, :]                                       # [T,B,C,129]
    q1 = scq.reshape(-1, D + 1) @ Wqs                             # [T*B*C,128]
    q1 = q1.reshape(T, B, C, D) + (ge @ Wqf)[None, :, None, :]
    q1 *= np.float32(A / np.sqrt(DH))
    # [B, d, q] with q = t*C + c
    q1 = q1.transpose(1, 3, 0, 2).reshape(B, D, Q)
    q1ta = np.zeros((B, 128, Q), np.float32)
    q1tb = np.zeros((B, 128, Q), np.float32)
    for g in range(4):
        q1ta[:, 32 * g:32 * g + 16] = q1[:, 16 * HA[g]:16 * HA[g] + 16]
        q1tb[:, 32 * g:32 * g + 16] = q1[:, 16 * HB[g]:16 * HB[g] + 16]
    q1ta = q1ta.astype(BF)
    q1tb = q1tb.astype(BF)

    # ---- masks ----
    m4 = mask[:, :, :, 0, :]                                      # [T,B,C,N]
    # attention maskB: [B, 128 n', 4 j, Q] bf16, n = 128j + n'
    mt = m4.transpose(1, 3, 0, 2).reshape(B, N, Q)
    mb = np.where(mt, np.float32(MASKB), np.float32(BEXP))
    maskB = np.ascontiguousarray(
        mb.reshape(B, 4, 128, Q).transpose(0, 2, 1, 3)).astype(BF)
    # final-stage mask [B, 128 q', 4 i, N] bf16, value MNEG
    mn = m4.transpose(1, 0, 2, 3).reshape(B, Q, N)
    mkneg = np.ascontiguousarray(
        (mn.reshape(B, 4, 128, N).transpose(0, 2, 1, 3).astype(np.float32)
         * np.float32(MNEG))).astype(BF)

    # ---- weights ----
    Wk1 = np.asarray(inputs["Wk1"], np.float32)
    bfw = lambda x: np.ascontiguousarray(x).astype(BF)
    selZ = np.zeros((128, 128), np.float32)
    for p in range(128):
        selZ[32 * (p // 32) + 16, p] = 1.0
    weights = {
        "wk1a": bfw(_perm_cols(Wk1, HA)), "wk1b": bfw(_perm_cols(Wk1, HB)),
        "wv": bfw(inputs["Wv"]),
        "wk2s": bfw(np.asarray(inputs["Wk2"], np.float32)
                    / np.float32(np.sqrt(D))),
        "wouta": bfw(_perm_rows(np.asarray(inputs["Wout"], np.float32), HA)),
        "woutb": bfw(_perm_rows(np.asarray(inputs["Wout"], np.float32), HB)),
        "selz": selZ.astype(BF),
        # p4sel[q', t'] = 1 iff q'//4 == t'  (c-sum per step)
        "p4sel": np.stack([
            ((np.arange(128) // 4) == tp).astype(np.float32)
            for tp in range(32)], axis=1).astype(BF),
        # p432[t', q'] = -1 iff q'//4 == t'  (negated lnZ broadcast)
        "p432": (-np.stack([
            ((np.arange(128) // 4) == tp).astype(np.float32)
            for tp in range(32)])).astype(BF),
    }

    core_ins = []
    for ci in range(NCORES):
        b0 = ci * NB
        sl = slice(b0, b0 + NB)
        m = dict(weights)
        m.update({
            "net": np.ascontiguousarray(net[sl]),
            "q1ta": np.ascontiguousarray(q1ta[sl]),
            "q1tb": np.ascontiguousarray(q1tb[sl]),
            "maskb": np.ascontiguousarray(maskB[sl]),
            "mkneg": np.ascontiguousarray(mkneg[sl]),
        })
        core_ins.append(m)
    return core_ins


def build_kernel(nb=NB):
    import concourse.bacc as bacc
    import concourse.mybir as mybir
    import concourse.tile as tile

    dt = mybir.dt
    f32, bf16, i16, i32 = dt.float32, dt.bfloat16, dt.int16, dt.int32
    AF = mybir.ActivationFunctionType
    OP = mybir.AluOpType

    nc = bacc.Bacc("TRN2", target_bir_lowering=False, debug=False,
                   num_devices=NCORES)

    din = {}
    def dram(name, shape, dtype, kind="ExternalInput"):
        din[name] = nc.dram_tensor(name, shape, dtype, kind=kind)
        return din[name]

    net = dram("net", [nb, D, N], bf16)
    q1ta = dram("q1ta", [nb, 128, Q], bf16)
    q1tb = dram("q1tb", [nb, 128, Q], bf16)
    maskb = dram("maskb", [nb, 128, 4, Q], bf16)
    mkneg = dram("mkneg", [nb, 128, 4, N], bf16)
    for w in ("wk1a", "wk1b", "wv", "wk2s", "wouta", "woutb", "selz",
              "p4sel", "p432"):
        shape = ([128, 32] if w == "p4sel" else
                 ([32, 128] if w == "p432" else [128, 128]))
        dram(w, shape, bf16)
    # device layout [q'=(t', c), b, i, n]; host reassembles t = 32*i + t'
    out = dram("out", [128, nb, 4, N], bf16, kind="ExternalOutput")

    with tile.TileContext(nc) as tc:
        from contextlib import ExitStack
        with ExitStack() as ctx:
            wp = ctx.enter_context(tc.tile_pool(name="wp", bufs=1))
            io = ctx.enter_context(tc.tile_pool(name="io", bufs=2))
            wk = ctx.enter_context(tc.tile_pool(name="wk", bufs=3))
            big = ctx.enter_context(tc.tile_pool(name="big", bufs=3))
            ese = ctx.enter_context(tc.tile_pool(name="ese", bufs=4))
            sm = ctx.enter_context(tc.tile_pool(name="sm", bufs=3))
            # PSUM: pss 2 x [128,2,512] (2 banks each) + pu 4 x [128,512]
            pss = ctx.enter_context(tc.tile_pool(name="pss", bufs=2, space="PSUM"))
            pu = ctx.enter_context(tc.tile_pool(name="pu", bufs=4, space="PSUM"))

            W = {}
            for wn in ("wk1a", "wk1b", "wv", "wk2s", "wouta", "woutb",
                       "selz", "p4sel", "p432"):
                t = wp.tile(list(din[wn].shape), din[wn].dtype, tag=f"w_{wn}")
                nc.sync.dma_start(out=t, in_=din[wn][:, :])
                W[wn] = t

            def stage0(b):
                """DMA, projections, attention; ends with u2 copies."""
                st = {}
                net_t = io.tile([D, N], bf16, tag="net")
                nc.sync.dma_start(out=net_t, in_=net[b])
                q1a_t = io.tile([128, Q], bf16, tag="q1a")
                nc.sync.dma_start(out=q1a_t, in_=q1ta[b])
                q1b_t = io.tile([128, Q], bf16, tag="q1b")
                nc.sync.dma_start(out=q1b_t, in_=q1tb[b])
                mb_t = io.tile([128, 4, Q], bf16, tag="maskb")
                nc.sync.dma_start(out=mb_t, in_=maskb[b])
                mkn_t = io.tile([128, 4, N], bf16, tag="mkneg")
                nc.sync.dma_start(out=mkn_t, in_=mkneg[b])
                st["mkn_t"] = mkn_t

                def proj_to_sbuf(wtile, rhs, tag):
                    ps = pu.tile([128, N], f32, name=f"p_{tag}", tag="pu")
                    nc.tensor.matmul(ps, lhsT=wtile, rhs=rhs)
                    sb = wk.tile([128, N], bf16, name=f"s_{tag}", tag=tag)
                    nc.scalar.copy(sb, ps)
                    return sb

                k1ta = proj_to_sbuf(W["wk1a"], net_t, "k1ta")
                k1tb = proj_to_sbuf(W["wk1b"], net_t, "k1tb")
                st["k2t"] = proj_to_sbuf(W["wk2s"], net_t, "k2t")

                pv = pu.tile([128, 4, 128], f32, tag="pu")
                for j in range(4):
                    nc.tensor.matmul(
                        pv[:, j, :], lhsT=net_t[:, 128 * j:128 * (j + 1)],
                        rhs=W["wv"])
                vaug = wk.tile([128, 2, 4, 128], bf16, tag="vaug")
                nc.gpsimd.memset(vaug, 0.0)
                nc.gpsimd.memset(
                    vaug.rearrange("p x j (g c) -> p x j g c", g=4)
                    [:, :, :, :, 16:17], 1.0)
                nc.scalar.copy(
                    vaug.rearrange("p x j (g c) -> p x j g c", g=4)
                    [:, :, :, :, 0:16],
                    pv.rearrange("p j (x g r) -> p x j g r", x=2, g=4))

                for pi, (k1t, q1t) in enumerate(
                        ((k1ta, q1a_t), (k1tb, q1b_t))):
                    psu = pu.tile([128, Q], f32, name=f"psu{pi}", tag="pu")
                    for j in range(4):
                        psA = pss.tile([128, 2, Q], f32, tag="sc")
                        psB = pss.tile([128, 2, Q], f32, tag="sc")
                        for g in range(4):
                            ps2 = psA if g < 2 else psB
                            sl = slice(32 * g, 32 * g + 16)
                            nc.tensor.matmul(
                                ps2[:, g % 2, :],
                                lhsT=k1t[sl, 128 * j:128 * (j + 1)],
                                rhs=q1t[sl, :], start=True, stop=True,
                                tile_position=(32 * g, 0),
                                skip_group_check=True)
                        mbb = mb_t[:, j, None, :].broadcast_to([128, 2, Q])
                        for half, ps2 in ((0, psA), (1, psB)):
                            u = (pi * 4 + j) * 2 + half
                            es = ese.tile([128, 2, Q], i16, tag="es")
                            if u in S_UNITS:
                                sp = ese.tile([128, 2, Q], bf16, tag="sp")
                                nc.scalar.copy(sp, ps2)
                                nc.vector.tensor_tensor(es, sp, mbb, OP.add)
                            else:
                                nc.vector.tensor_tensor(es, ps2, mbb, OP.add)
                            em = es.bitcast(bf16)
                            for g2 in (0, 1):
                                g = half * 2 + g2
                                nc.tensor.matmul(
                                    psu[32 * g:32 * g + 32, :],
                                    lhsT=vaug[:, pi, j, 32 * g:32 * g + 32],
                                    rhs=em[:, g2, :],
                                    start=(j == 0), stop=(j == 3),
                                    tile_position=(0, 32 * g),
                                    skip_group_check=True)
                    u2 = wk.tile([128, Q], bf16, name=f"u2_{pi}",
                                 tag=f"u2{pi}")
                    nc.scalar.copy(u2, psu)
                    st[f"u2{pi}"] = u2
                return st

            def stage1(st):
                """Normalize, Q2, logits, tanh."""
                un = {}
                for pi in range(2):
                    u2 = st[f"u2{pi}"]
                    zbc = pu.tile([128, Q], f32, name=f"zbc{pi}", tag="pu")
                    nc.tensor.matmul(zbc, lhsT=W["selz"], rhs=u2)
                    rinv = big.tile([128, Q], f32, name=f"rinv{pi}",
                                    tag=f"rinv{pi}")
                    nc.vector.reciprocal_approx_fast(out=rinv, in_=zbc)
                    u_n = wk.tile([128, Q], bf16, name=f"un_{pi}",
                                  tag=f"un{pi}")
                    nc.gpsimd.tensor_tensor(u_n, u2, rinv, OP.mult)
                    un[pi] = u_n

                pq2 = pu.tile([128, Q], f32, tag="pu")
                nc.tensor.matmul(pq2, lhsT=W["wouta"], rhs=un[0],
                                 start=True, stop=False)
                nc.tensor.matmul(pq2, lhsT=W["woutb"], rhs=un[1],
                                 start=False, stop=True)
                q2t = wk.tile([128, Q], bf16, tag="q2t")
                nc.scalar.copy(q2t, pq2)

                th = big.tile([128, 4, N], bf16, tag="th")
                for i in range(4):
                    pl = pu.tile([128, N], f32, name=f"pl{i}", tag="pu")
                    nc.tensor.matmul(
                        pl, lhsT=q2t[:, 128 * i:128 * (i + 1)],
                        rhs=st["k2t"])
                    nc.scalar.activation(th[:, i, :], pl, AF.Tanh)
                st["th"] = th
                return st

            def stage2(st):
                """Mask, Z, lnZ, output."""
                th, mkn_t = st["th"], st["mkn_t"]
                s1 = big.tile([128, 4, N], bf16, tag="s1")
                nc.vector.tensor_tensor(s1, th, mkn_t, OP.add)
                es2 = big.tile([128, 4, N], i16, tag="es2")
                nc.gpsimd.tensor_scalar(es2, s1, A10, BEXP, OP.mult, OP.add)
                zf = sm.tile([128, 4], f32, tag="zf")
                nc.vector.tensor_reduce(zf, es2.bitcast(bf16),
                                        mybir.AxisListType.X, OP.add)
                zb = sm.tile([128, 4], bf16, tag="zb")
                nc.vector.tensor_copy(zb, zf)
                pmisc = pu.tile([128, N], f32, tag="pu")
                nc.tensor.matmul(pmisc[0:32, 0:4], lhsT=W["p4sel"], rhs=zb)
                zi = sm.tile([32, 4], f32, tag="zi")
                nc.vector.tensor_copy(zi, pmisc[0:32, 0:4].bitcast(i32))
                lnzb = sm.tile([32, 4], bf16, tag="lnzb")
                nc.vector.tensor_scalar(
                    lnzb, zi, LN2 / (1 << 23),
                    -(127.043 + SCHRAUD_MEANLOG) * LN2, OP.mult, OP.add)
                nc.tensor.matmul(pmisc[:, 4:8], lhsT=W["p432"], rhs=lnzb)
                bias = sm.tile([128, 4], f32, tag="bias")
                nc.vector.tensor_copy(bias, pmisc[:, 4:8])

                out_sb = big.tile([128, 4, N], bf16, tag="outsb")
                for i in range(4):
                    nc.gpsimd.tensor_scalar(
                        out_sb[:, i, :], s1[:, i, :], 10.0,
                        bias[:, i:i + 1], OP.mult, OP.add)
                nc.sync.dma_start(out=out[:, st["b"], :, :], in_=out_sb)

            # 3-stage software pipeline with 1-batch skew per stage
            inflight = {}
            for b in range(nb + 2):
                if b < nb:
                    inflight[b] = stage0(b)
                    inflight[b]["b"] = b
                if b >= 1 and (b - 1) in inflight and b - 1 < nb:
                    stage1(inflight[b - 1])
                if b >= 2:
                    stage2(inflight.pop(b - 2))

    nc.compile()
    return nc


_CACHED = None


def _get_nc():
    global _CACHED
    if _CACHED is None:
        _CACHED = build_kernel()
    return _CACHED


def kernel(**inputs):
    from concourse.bass_utils import run_bass_kernel_spmd

    core_ins = _host_prep(inputs)
    nc = _get_nc()
    res = run_bass_kernel_spmd(nc, core_ins, core_ids=list(range(NCORES)))
    outs = [_unscramble(r["out"]) for r in res.results]   # each [T, NB, 2048]
    return np.concatenate(outs, axis=1)                   # [T, B, 2048]


def _unscramble(dev):
    """Device [128 q'=(t',c), nb, 4 i, 512 n] -> [T, nb, C*N], t=32i+t'."""
    nb = dev.shape[1]
    return (dev.astype(np.float32)
            .reshape(32, C, nb, 4, N)
            .transpose(3, 0, 2, 1, 4)
            .reshape(T, nb, C * N))


# revision 8
# speedup vs baseline: 1.3417x; 1.0995x over previous
"""Trainium2 Bass kernel for nn_DecoderCell (B=128,N=512,C=4,T=128,D=128,H=8).

Pure data-parallel over batch B across 8 NeuronCores (16 b/core).

v2 design notes:
- Q1 (query projection incl. step context + fixed query) precomputed on host,
  pre-scaled by A/sqrt(DH) with A = 2^7/ln2, head-permuted into two pass
  tiles (heads 0-3 / 4-7), so score PSUM holds A*s directly.
- Softmax exp is a Schraudolph bit-trick: one tensor_tensor(psum + maskB ->
  int16) per score tile; maskB carries the exponent bias 16256 for kept
  entries and -40960 for masked ones, which saturates the int16 convert to
  -32768 = bf16 -0.0 (exact zero contribution to U and Z).
- Score tiles split between DVE (TT direct, 1x) and ScalarE (Identity copy
  to bf16, then DVE TT at 2x) to balance the PSUM-read bottleneck across
  both PSUM-capable engines. GpSimd (no PSUM access) takes SBUF-only work:
  vaug memsets, final-stage Schraudolph exp and output scale/bias.
- Z rows come free from a ones-augmented V; broadcast via one select-matmul
  (selZ) straight from the u2 copy, then reciprocal_approx_fast + multiply.
- Final stage: tanh (ScalarE) -> +mask (DVE 2x) -> Schraudolph exp (GpSimd)
  -> row-sum Z (DVE) -> log via exponent-bits affine -> scale/bias out.
  lnZ affine includes the +0.0397 mean-log Schraudolph correction.
"""
import numpy as np
import ml_dtypes

D = 128
N = 512
C = 4
T = 128
Q = T * C          # 512 queries per batch, q = t*C + c
H = 8
DH = 16
NB = 16            # batches per core
NCORES = 8

A = 184.6649652337873       # 2^7/ln2
BEXP = 16256.0              # 127*128: bf16 exponent bias << 7
MASKB = -40960.0            # BEXP + mask -> saturate int16 -> bf16 -0.0
MNEG = -1.0e8               # final-stage mask (x10 ~= -1e9), bf16
A10 = 1846.649652337873     # 10*A for final exp
LN2 = 0.6931471805599453
SCHRAUD_MEANLOG = 0.0572809000084; # E[log2((1+f)/2^f)], f~U(0,1)

HA = [0, 1, 2, 3]
HB = [4, 5, 6, 7]
BF = ml_dtypes.bfloat16

# score-tile units 0..15: (pass, j, A/B) -> u = (pi*4+j)*2 + (0 if A else 1)
# s-units go through ScalarE copy + DVE 2x TT; rest are DVE 1x TT.
S_UNITS = frozenset({1, 3, 6, 8, 11, 13})


def _perm_cols(W, heads):
    """Columns of W[*,128] so head g sits at cols 32g..32g+15, zeros after."""
    out = np.zeros_like(W)
    for g, h in enumerate(heads):
        out[:, 32 * g:32 * g + 16] = W[:, 16 * h:16 * h + 16]
    return out


def _perm_rows(W, heads):
    out = np.zeros_like(W)
    for g, h in enumerate(heads):
        out[32 * g:32 * g + 16, :] = W[16 * h:16 * h + 16, :]
    return out


def _host_prep(inputs):
    """Full-input numpy prep -> per-core input dicts."""
    ne = np.ascontiguousarray(inputs["node_embeddings"], np.float32)
    ge = np.ascontiguousarray(inputs["graph_embedding"], np.float32)
    sc = np.ascontiguousarray(inputs["step_context"], np.float32)
    mask = np.asarray(inputs["mask"])
    B = ne.shape[0]

    net = ne.transpose(0, 2, 1).astype(BF)                        # [B,D,N]

    # ---- host Q1: (sc @ Wq_step + ge @ Wq_fixed) * (A/sqrt(DH)) ----
    Wqs = np.asarray(inputs["Wq_step"], np.float32)               # [129,128]
    Wqf = np.asarray(inputs["Wq_fixed"], np.float32)
    scq = sc[:, :, :, 0, :]                                       # [T,B,C,129]
    q1 = scq.reshape(-1, D + 1) @ Wqs                             # [T*B*C,128]
    q1 = q1.reshape(T, B, C, D) + (ge @ Wqf)[None, :, None, :]
    q1 *= np.float32(A / np.sqrt(DH))
    # [B, d, q] with q = t*C + c
    q1 = q1.transpose(1, 3, 0, 2).reshape(B, D, Q)
    q1ta = np.zeros((B, 128, Q), np.float32)
    q1tb = np.zeros((B, 128, Q), np.float32)
    for g in range(4):
        q1ta[:, 32 * g:32 * g + 16] = q1[:, 16 * HA[g]:16 * HA[g] + 16]
        q1tb[:, 32 * g:32 * g + 16] = q1[:, 16 * HB[g]:16 * HB[g] + 16]
    q1ta = q1ta.astype(BF)
    q1tb = q1tb.astype(BF)

    # ---- masks ----
    m4 = mask[:, :, :, 0, :]                                      # [T,B,C,N]
    # attention maskB: [B, 128 n', 4 j, Q] bf16, n = 128j + n'
    mt = m4.transpose(1, 3, 0, 2).reshape(B, N, Q)
    mb = np.where(mt, np.float32(MASKB), np.float32(BEXP))
    maskB = np.ascontiguousarray(
        mb.reshape(B, 4, 128, Q).transpose(0, 2, 1, 3)).astype(BF)
    # final-stage mask [B, 128 q', 4 i, N] bf16, value MNEG
    mn = m4.transpose(1, 0, 2, 3).reshape(B, Q, N)
    mkneg = np.ascontiguousarray(
        (mn.reshape(B, 4, 128, N).transpose(0, 2, 1, 3).astype(np.float32)
         * np.float32(MNEG))).astype(BF)

    # ---- weights ----
    Wk1 = np.asarray(inputs["Wk1"], np.float32)
    bfw = lambda x: np.ascontiguousarray(x).astype(BF)
    selZ = np.zeros((128, 128), np.float32)
    for p in range(128):
        selZ[32 * (p // 32) + 16, p] = 1.0
    weights = {
        "wk1a": bfw(_perm_cols(Wk1, HA)), "wk1b": bfw(_perm_cols(Wk1, HB)),
        "wv": bfw(inputs["Wv"]),
        "wk2s": bfw(np.asarray(inputs["Wk2"], np.float32)
                    / np.float32(np.sqrt(D))),
        "wouta": bfw(_perm_rows(np.asarray(inputs["Wout"], np.float32), HA)),
        "woutb": bfw(_perm_rows(np.asarray(inputs["Wout"], np.float32), HB)),
        "selz": selZ.astype(BF),
        # p4sel[q', t'] = 1 iff q'//4 == t'  (c-sum per step)
        "p4sel": np.stack([
            ((np.arange(128) // 4) == tp).astype(np.float32)
            for tp in range(32)], axis=1).astype(BF),
        # p432[t', q'] = -1 iff q'//4 == t'  (negated lnZ broadcast)
        "p432": (-np.stack([
            ((np.arange(128) // 4) == tp).astype(np.float32)
            for tp in range(32)])).astype(BF),
    }

    core_ins = []
    for ci in range(NCORES):
        b0 = ci * NB
        sl = slice(b0, b0 + NB)
        m = dict(weights)
        m.update({
            "net": np.ascontiguousarray(net[sl]),
            "q1ta": np.ascontiguousarray(q1ta[sl]),
            "q1tb": np.ascontiguousarray(q1tb[sl]),
            "maskb": np.ascontiguousarray(maskB[sl]),
            "mkneg": np.ascontiguousarray(mkneg[sl]),
        })
        core_ins.append(m)
    return core_ins


def build_kernel(nb=NB):
    import concourse.bacc as bacc
    import concourse.mybir as mybir
    import concourse.tile as tile

    dt = mybir.dt
    f32, bf16, i16, i32 = dt.float32, dt.bfloat16, dt.int16, dt.int32
    AF = mybir.ActivationFunctionType
    OP = mybir.AluOpType

    nc = bacc.Bacc("TRN2", target_bir_lowering=False, debug=False,
                   num_devices=NCORES)

    din = {}
    def dram(name, shape, dtype, kind="ExternalInput"):
        din[name] = nc.dram_tensor(name, shape, dtype, kind=kind)
        return din[name]

    net = dram("net", [nb, D, N], bf16)
    q1ta = dram("q1ta", [nb, 128, Q], bf16)
    q1tb = dram("q1tb", [nb, 128, Q], bf16)
    maskb = dram("maskb", [nb, 128, 4, Q], bf16)
    mkneg = dram("mkneg", [nb, 128, 4, N], bf16)
    for w in ("wk1a", "wk1b", "wv", "wk2s", "wouta", "woutb", "selz",
              "p4sel", "p432"):
        shape = ([128, 32] if w == "p4sel" else
                 ([32, 128] if w == "p432" else [128, 128]))
        dram(w, shape, bf16)
    # device layout [q'=(t', c), b, i, n]; host reassembles t = 32*i + t'
    out = dram("out", [128, nb, 4, N], bf16, kind="ExternalOutput")

    with tile.TileContext(nc) as tc:
        from contextlib import ExitStack
        with ExitStack() as ctx:
            wp = ctx.enter_context(tc.tile_pool(name="wp", bufs=1))
            io = ctx.enter_context(tc.tile_pool(name="io", bufs=3))
            wk = ctx.enter_context(tc.tile_pool(name="wk", bufs=3))
            big = ctx.enter_context(tc.tile_pool(name="big", bufs=3))
            ese = ctx.enter_context(tc.tile_pool(name="ese", bufs=4))
            sm = ctx.enter_context(tc.tile_pool(name="sm", bufs=3))
            # PSUM budget (8 banks): pss "sc" ring2 x 2 banks = 4;
            # pu "hold" ring2 x 1 bank (U accum) = 2; pu "flow" ring2 = 2.
            pss = ctx.enter_context(tc.tile_pool(name="pss", bufs=2, space="PSUM"))
            pu = ctx.enter_context(tc.tile_pool(name="pu", bufs=2, space="PSUM"))

            W = {}
            for wn in ("wk1a", "wk1b", "wv", "wk2s", "wouta", "woutb",
                       "selz", "p4sel", "p432"):
                t = wp.tile(list(din[wn].shape), din[wn].dtype, tag=f"w_{wn}")
                nc.sync.dma_start(out=t, in_=din[wn][:, :])
                W[wn] = t

            def stage0(b, fillers):
                """DMA, projections, attention; runs filler chunks from
                older batches between attention steps to keep queues fed."""
                st = {"b": b}

                def fill(n=2):
                    for _ in range(n):
                        if fillers:
                            fillers.pop(0)()

                net_t = io.tile([D, N], bf16, tag="net")
                nc.sync.dma_start(out=net_t, in_=net[b])
                q1a_t = io.tile([128, Q], bf16, tag="q1a")
                nc.sync.dma_start(out=q1a_t, in_=q1ta[b])
                q1b_t = io.tile([128, Q], bf16, tag="q1b")
                nc.sync.dma_start(out=q1b_t, in_=q1tb[b])
                mb_t = io.tile([128, 4, Q], bf16, tag="maskb")
                nc.sync.dma_start(out=mb_t, in_=maskb[b])
                mkn_t = io.tile([128, 4, N], bf16, tag="mkneg")
                nc.sync.dma_start(out=mkn_t, in_=mkneg[b])
                st["mkn_t"] = mkn_t

                def proj_to_sbuf(wtile, rhs, tag):
                    ps = pu.tile([128, N], f32, name=f"p_{tag}", tag="flow")
                    nc.tensor.matmul(ps, lhsT=wtile, rhs=rhs)
                    sb = wk.tile([128, N], bf16, name=f"s_{tag}", tag=tag)
                    nc.scalar.copy(sb, ps)
                    return sb

                k1ta = proj_to_sbuf(W["wk1a"], net_t, "k1ta")
                k1tb = proj_to_sbuf(W["wk1b"], net_t, "k1tb")
                st["k2t"] = proj_to_sbuf(W["wk2s"], net_t, "k2t")

                pv = pu.tile([128, 4, 128], f32, tag="flow")
                for j in range(4):
                    nc.tensor.matmul(
                        pv[:, j, :], lhsT=net_t[:, 128 * j:128 * (j + 1)],
                        rhs=W["wv"])
                vaug = wk.tile([128, 2, 4, 128], bf16, tag="vaug")
                nc.gpsimd.memset(vaug, 0.0)
                nc.gpsimd.memset(
                    vaug.rearrange("p x j (g c) -> p x j g c", g=4)
                    [:, :, :, :, 16:17], 1.0)
                nc.scalar.copy(
                    vaug.rearrange("p x j (g c) -> p x j g c", g=4)
                    [:, :, :, :, 0:16],
                    pv.rearrange("p j (x g r) -> p x j g r", x=2, g=4))

                psu = {}
                pending_u = None  # (pi, j, em) deferred by one step
                u2 = {}

                for pi, (k1t, q1t) in enumerate(
                        ((k1ta, q1a_t), (k1tb, q1b_t))):
                    psu[pi] = pu.tile([128, Q], f32, name=f"psu{pi}",
                                      tag="hold")
                    for j in range(4):
                        psA = pss.tile([128, 2, Q], f32, tag="sc")
                        psB = pss.tile([128, 2, Q], f32, tag="sc")
                        for g in range(4):
                            ps2 = psA if g < 2 else psB
                            sl = slice(32 * g, 32 * g + 16)
                            nc.tensor.matmul(
                                ps2[:, g % 2, :],
                                lhsT=k1t[sl, 128 * j:128 * (j + 1)],
                                rhs=q1t[sl, :], start=True, stop=True,
                                tile_position=(32 * g, 0),
                                skip_group_check=True)
                        mbb = mb_t[:, j, None, :].broadcast_to([128, 2, Q])
                        ems = []
                        for half, ps2 in ((0, psA), (1, psB)):
                            u = (pi * 4 + j) * 2 + half
                            es = ese.tile([128, 2, Q], i16, tag="es")
                            if u in S_UNITS:
                                sp = ese.tile([128, 2, Q], bf16, tag="sp")
                                nc.scalar.copy(sp, ps2)
                                nc.vector.tensor_tensor(es, sp, mbb, OP.add)
                            else:
                                nc.vector.tensor_tensor(es, ps2, mbb, OP.add)
                            ems.append(es.bitcast(bf16))
                        if pending_u is not None:
                            ppi, pj, pems = pending_u
                            for half in (0, 1):
                                for g2 in (0, 1):
                                    g = half * 2 + g2
                                    nc.tensor.matmul(
                                        psu[ppi][32 * g:32 * g + 32, :],
                                        lhsT=vaug[:, ppi, pj,
                                                  32 * g:32 * g + 32],
                                        rhs=pems[half][:, g2, :],
                                        start=(pj == 0), stop=(pj == 3),
                                        tile_position=(0, 32 * g),
                                        skip_group_check=True)
                            if pj == 3:
                                u2[ppi] = wk.tile([128, Q], bf16,
                                                  name=f"u2_{ppi}",
                                                  tag=f"u2{ppi}")
                                nc.scalar.copy(u2[ppi], psu[ppi])
                        pending_u = (pi, j, ems)
                        fill(2)
                # drain last step's U
                ppi, pj, pems = pending_u
                for half in (0, 1):
                    for g2 in (0, 1):
                        g = half * 2 + g2
                        nc.tensor.matmul(
                            psu[ppi][32 * g:32 * g + 32, :],
                            lhsT=vaug[:, ppi, pj, 32 * g:32 * g + 32],
                            rhs=pems[half][:, g2, :],
                            start=(pj == 0), stop=(pj == 3),
                            tile_position=(0, 32 * g),
                            skip_group_check=True)
                u2[1] = wk.tile([128, Q], bf16, name="u2_1b", tag="u21")
                nc.scalar.copy(u2[1], psu[1])
                st["u20"], st["u21"] = u2[0], u2[1]
                return st

            def make_chunks(st):
                """S1+S2 of batch st: list of closures (FIFO order)."""
                ch = []
                un = {}
                rv = {}

                def mk_zbc(pi):
                    def f():
                        u2 = st[f"u2{pi}"]
                        zbc = pu.tile([128, Q], f32, name=f"zbc{pi}",
                                      tag="flow")
                        nc.tensor.matmul(zbc, lhsT=W["selz"], rhs=u2)
                        rinv = big.tile([128, Q], f32, name=f"rinv{pi}",
                                        tag=f"rinv{pi}")
                        nc.vector.reciprocal_approx_fast(out=rinv, in_=zbc)
                        rv[pi] = rinv
                    return f
                ch.append(mk_zbc(0))
                ch.append(mk_zbc(1))

                def c_un():
                    for pi in range(2):
                        u_n = wk.tile([128, Q], bf16, name=f"un_{pi}",
                                      tag=f"un{pi}")
                        nc.gpsimd.tensor_tensor(u_n, st[f"u2{pi}"], rv[pi],
                                                OP.mult)
                        un[pi] = u_n
                ch.append(c_un)

                def c_q2():
                    pq2 = pu.tile([128, Q], f32, tag="flow")
                    nc.tensor.matmul(pq2, lhsT=W["wouta"], rhs=un[0],
                                     start=True, stop=False)
                    nc.tensor.matmul(pq2, lhsT=W["woutb"], rhs=un[1],
                                     start=False, stop=True)
                    q2t = wk.tile([128, Q], bf16, tag="q2t")
                    nc.scalar.copy(q2t, pq2)
                    st["q2t"] = q2t
                ch.append(c_q2)

                th = None

                def mk_logit(i0):
                    def f():
                        nonlocal th
                        if th is None:
                            th = big.tile([128, 4, N], bf16, name="th",
                                          tag="th")
                            st["th"] = th
                        for i in (i0, i0 + 1):
                            pl = pu.tile([128, N], f32, name=f"pl{i}",
                                         tag="flow")
                            nc.tensor.matmul(
                                pl, lhsT=st["q2t"][:, 128 * i:128 * (i + 1)],
                                rhs=st["k2t"])
                            nc.scalar.activation(th[:, i, :], pl, AF.Tanh)
                    return f
                ch.append(mk_logit(0))
                ch.append(mk_logit(2))

                s1es = {}

                def c_s1():
                    s1 = big.tile([128, 4, N], bf16, tag="s1")
                    nc.vector.tensor_tensor(s1, st["th"], st["mkn_t"],
                                            OP.add)
                    es2 = big.tile([128, 4, N], i16, tag="es2")
                    nc.gpsimd.tensor_scalar(es2, s1, A10, BEXP,
                                            OP.mult, OP.add)
                    s1es["s1"], s1es["es2"] = s1, es2
                ch.append(c_s1)

                def c_z():
                    zf = sm.tile([128, 4], f32, tag="zf")
                    nc.vector.tensor_reduce(zf, s1es["es2"].bitcast(bf16),
                                            mybir.AxisListType.X, OP.add)
                    zb = sm.tile([128, 4], bf16, tag="zb")
                    nc.vector.tensor_copy(zb, zf)
                    s1es["zb"] = zb
                ch.append(c_z)

                def c_lnz():
                    pmisc = pu.tile([128, N], f32, tag="flow")
                    nc.tensor.matmul(pmisc[0:32, 0:4], lhsT=W["p4sel"],
                                     rhs=s1es["zb"])
                    zi = sm.tile([32, 4], f32, tag="zi")
                    nc.vector.tensor_copy(zi, pmisc[0:32, 0:4].bitcast(i32))
                    lnzb = sm.tile([32, 4], bf16, tag="lnzb")
                    nc.vector.tensor_scalar(
                        lnzb, zi, LN2 / (1 << 23),
                        -(127.043 + SCHRAUD_MEANLOG) * LN2, OP.mult, OP.add)
                    nc.tensor.matmul(pmisc[:, 4:8], lhsT=W["p432"],
                                     rhs=lnzb)
                    bias = sm.tile([128, 4], f32, tag="bias")
                    nc.vector.tensor_copy(bias, pmisc[:, 4:8])
                    s1es["bias"] = bias
                ch.append(c_lnz)

                def c_out():
                    out_sb = big.tile([128, 4, N], bf16, tag="outsb")
                    for i in range(4):
                        nc.gpsimd.tensor_scalar(
                            out_sb[:, i, :], s1es["s1"][:, i, :], 10.0,
                            s1es["bias"][:, i:i + 1], OP.mult, OP.add)
                    nc.sync.dma_start(out=out[:, st["b"], :, :], in_=out_sb)
                ch.append(c_out)
                return ch

            pend = []
            for b in range(nb):
                st = stage0(b, pend)
                pend.extend(make_chunks(st))
            for f in pend:
                f()

    nc.compile()
    return nc


_CACHED = None


def _get_nc():
    global _CACHED
    if _CACHED is None:
        _CACHED = build_kernel()
    return _CACHED


def kernel(**inputs):
    from concourse.bass_utils import run_bass_kernel_spmd

    core_ins = _host_prep(inputs)
    nc = _get_nc()
    res = run_bass_kernel_spmd(nc, core_ins, core_ids=list(range(NCORES)))
    outs = [_unscramble(r["out"]) for r in res.results]   # each [T, NB, 2048]
    return np.concatenate(outs, axis=1)                   # [T, B, 2048]


def _unscramble(dev):
    """Device [128 q'=(t',c), nb, 4 i, 512 n] -> [T, nb, C*N], t=32i+t'."""
    nb = dev.shape[1]
    return (dev.astype(np.float32)
            .reshape(32, C, nb, 4, N)
            .transpose(3, 0, 2, 1, 4)
            .reshape(T, nb, C * N))
